# revision 19
# baseline (speedup 1.0000x reference)
"""Multi-head attention kernel for Trainium2, sharded over 8 NeuronCores.

Problem: B=2, S=2048, D=1024, H=16 heads (d_k=64), fp32 in/out, mask == all-ones.

Sharding: 2 heads per core (head/tensor parallel). Each core computes, for its
128-wide slice `sl` of the projection output dims:
    QT/KT = (W_*[sl] @ x.T)         -> [128, 4096]  (transposed layout, bf16)
    V     = x @ W_V[sl].T chunks    -> vaug tiles [krows, dims|ones]
    per (b, head): scoresT = K_h @ Q_h.T (contraction d_k=64, row-tiled pair)
    P.T = exp(scoresT / 8)          (no max-subtraction needed: |scores| < 8)
    acc_h = [V_h | 1].T @ P.T       -> psum; trailing row = softmax denom
    outT = acc * recip(denom) via fast-reciprocal + K=1 PE broadcast matmuls
    partialT = woT.T @ outT         -> partial.T [1024, 4096] bf16
Host: sums the 8 partials (fp32), transposes, reshapes.

Scheduling: scores/exp/attnV are software-pipelined (attnV one k-chunk behind
scores); independent matmul work (other-batch projections, V chunks, deferred
output projection) is pumped one unit per k-chunk to keep the PE continuously
busy while ACT computes exp.
"""
import sys

sys.path.insert(0, "/opt/trn_rl_repo")

from contextlib import ExitStack

import ml_dtypes
import numpy as np

import concourse.bass as bass
from concourse import bacc
import concourse.mybir as mybir
import concourse.tile as tile
from concourse.bass_utils import run_bass_kernel_spmd

BF16 = ml_dtypes.bfloat16
D = 1024
B = 2
S = 2048
BS = B * S            # 4096 rows
N_CORES = 8
SLW = D // N_CORES    # 128 output dims per core (2 heads x 64)
DK = 64
KC = S // 128         # 16 k-chunks per batch
QB = S // 512         # 4 q-blocks of 512 per batch
W1 = 97               # head-1 acc width: 64 data + 32 pad + denom at row 96
F32 = mybir.dt.float32
BF = mybir.dt.bfloat16

_nc_cache = {}
DEBUG_DUMPS = False  # when True, adds intermediate-dump outputs (dev only)


def _build_program():
    nc = bacc.Bacc("TRN2", target_bir_lowering=False, debug=False, num_devices=8)
    xT = nc.dram_tensor("xT", [D, BS], BF, kind="ExternalInput")
    wqT = nc.dram_tensor("wqT", [D, SLW], BF, kind="ExternalInput")
    wkT = nc.dram_tensor("wkT", [D, SLW], BF, kind="ExternalInput")
    wvT = nc.dram_tensor("wvT", [D, SLW], BF, kind="ExternalInput")
    woT = nc.dram_tensor("woT", [SLW, D], BF, kind="ExternalInput")
    out = nc.dram_tensor("out", [D, BS], BF, kind="ExternalOutput")

    with tile.TileContext(nc) as tc, ExitStack() as ctx:
        _emit(ctx, tc, nc, xT, wqT, wkT, wvT, woT, out)
    nc.compile()
    return nc


def _emit(ctx, tc, nc, xT, wqT, wkT, wvT, woT, out):
    Exp = mybir.ActivationFunctionType.Exp

    consts = ctx.enter_context(tc.tile_pool(name="consts", bufs=1))
    big = ctx.enter_context(tc.tile_pool(name="big", bufs=1))
    stage = ctx.enter_context(tc.tile_pool(name="stage", bufs=2))
    small = ctx.enter_context(tc.tile_pool(name="small", bufs=2))
    pt_pool = ctx.enter_context(tc.tile_pool(name="pt", bufs=3))
    # PSUM (8 banks): scores 2 x [128,1024] = 4 banks; attnV accumulators
    # (held across each q-block's k loop) 2 banks; shared work ring (V/proj
    # psum, output-proj matmuls, reciprocal broadcasts) 2 banks.
    ps_sc = ctx.enter_context(tc.tile_pool(name="ps_sc", bufs=2, space="PSUM"))
    ps_acc = ctx.enter_context(tc.tile_pool(name="ps_acc", bufs=1, space="PSUM"))
    ps_work = ctx.enter_context(tc.tile_pool(name="ps_work", bufs=2, space="PSUM"))

    # ---- Input DMAs: wk/wq first (first projections), x in 32 column-major
    # pieces spread over 4 engine queues so the first projection block is
    # ready ~3us in; wv/wo ordered later on queues that are past their
    # critical pieces.
    w_sbs = {}
    for name, w in (("k", wkT), ("q", wqT), ("v", wvT)):
        w_sbs[name] = consts.tile([128, 8 * SLW], BF, tag=f"w{name}", name=f"w{name}")
    for name in ("k", "q"):
        nc.sync.dma_start(
            w_sbs[name][:].rearrange("p (c m) -> p c m", c=8),
            {"k": wkT, "q": wqT}[name][:].rearrange("(c p) m -> p c m", p=128),
        )
    woT_sb = consts.tile([128, D], BF, tag="wo")

    xt_sb = [big.tile([128, BS], BF, tag=f"xt{c}", name=f"xt{c}") for c in range(8)]
    engines = [nc.sync, nc.scalar, nc.gpsimd]
    idx = 0
    for p in range(4):
        for c in range(8):
            cols = slice(p * 1024, (p + 1) * 1024)
            engines[idx % 3].dma_start(xt_sb[c][:, cols], xT[c * 128 : (c + 1) * 128, cols])
            idx += 1
        if p == 0:
            nc.scalar.dma_start(
                w_sbs["v"][:].rearrange("p (c m) -> p c m", c=8),
                wvT[:].rearrange("(c p) m -> p c m", p=128),
            )
        if p == 1:
            nc.sync.dma_start(woT_sb[:], woT[:])

    # ---- Constants: vaug ones-columns (memset 1.0; V copies overwrite data
    # cols, pad cols harmlessly stay 1), K=1 broadcast rows at partitions
    # 64 and 96 (legal PE tile_position rows).
    vaug0 = consts.tile([128, KC * B * 65], BF, tag="vaug0", name="vaug0")
    vaug1 = consts.tile([128, KC * B * W1], BF, tag="vaug1", name="vaug1")
    nc.gpsimd.memset(vaug0[:], 1.0)
    nc.gpsimd.memset(vaug1[:], 1.0)
    ones_t = consts.tile([128, 64], BF, tag="ones_t", name="ones_t")
    nc.gpsimd.memset(ones_t[64:65, :], 1.0)
    nc.gpsimd.memset(ones_t[96:97, :], 1.0)
    # reciprocal_approx_fast mis-executes on single-partition slices on HW,
    # so the denominators ride in rows 64/96 of a full-height tile and the
    # reciprocal runs over all 128 partitions (same cycle count — cost is
    # free-dim size). Unused rows hold 1.0 so the op stays in-range.
    den_t = consts.tile([128, 512], F32, tag="den_t", name="den_t")
    rec_t = consts.tile([128, 512], F32, tag="rec_t", name="rec_t")
    nc.gpsimd.memset(den_t[:], 1.0)

    proj = {}
    for name in ("k", "q"):
        proj[name] = big.tile([128, BS], BF, tag=f"{name}T", name=f"{name}T")

    def emit_qk_block(name, n):
        # n: global 512-col block index (0..7) over both batches.
        dst = proj[name]
        w_sb = w_sbs[name]
        ps = ps_work.tile([128, 512], F32, tag="work", name=f"ps_{name}{n}")
        for d in range(8):
            nc.tensor.matmul(
                ps[:],
                w_sb[:, bass.ts(d, SLW)],
                xt_sb[d][:, bass.ts(n, 512)],
                start=(d == 0),
                stop=(d == 7),
            )
        nc.vector.tensor_copy(dst[:, bass.ts(n, 512)], ps[:])

    def emit_v_chunk(rc):
        # rc: global 128-row chunk (0..31). vaug0[:, rc*65+m] = V[rc*128+p, m],
        # vaug1[:, rc*97+m] = V[rc*128+p, 64+m]; ones cols feed the denom rows.
        wv_sb = w_sbs["v"]
        ps = ps_work.tile([128, 128], F32, tag="work", name=f"ps_v{rc}")
        for d in range(8):
            nc.tensor.matmul(
                ps[:],
                xt_sb[d][:, bass.ts(rc, 128)],
                wv_sb[:, bass.ts(d, SLW)],
                start=(d == 0),
                stop=(d == 7),
            )
        # GPSIMD can't access PSUM, so these casts stay on DVE.
        nc.vector.tensor_copy(vaug0[:, rc * 65 : rc * 65 + 64], ps[:, 0:64])
        nc.vector.tensor_copy(vaug1[:, rc * W1 : rc * W1 + 64], ps[:, 64:128])

    # ---- Filler units: independent PE work pumped one unit per k-chunk so
    # the PE never head-of-line blocks on ACT's exp. Units carry keys so
    # consumers can force ("ensure") their producers to emit first.
    filler = []  # list of (key, fn)

    def pump():
        if filler:
            filler.pop(0)[1]()

    def ensure(key):
        for i, (k, fn) in enumerate(filler):
            if k == key:
                filler.pop(i)
                fn()
                return

    # ---- Output projection, one q-block of one batch = 8 jc units + 1 DMA.
    def push_oproj(b, qb, outT):
        st = stage.tile([128, 8 * 512], BF, tag="st", name=f"st{b}_{qb}")

        def mk(jc):
            def unit():
                pm = ps_work.tile([128, 512], F32, tag="work", name=f"pm{b}_{qb}_{jc}")
                nc.tensor.matmul(
                    pm[:],
                    woT_sb[:, bass.ts(jc, 128)],
                    outT[:, bass.ts(qb, 512)],
                    start=True,
                    stop=True,
                )
                nc.vector.tensor_copy(st[:, bass.ts(jc, 512)], pm[:])
                if jc == 7:
                    cols = slice(b * S + qb * 512, b * S + (qb + 1) * 512)
                    nc.sync.dma_start(
                        out[:, cols].rearrange("(c p) m -> p c m", p=128),
                        st[:].rearrange("p (c m) -> p c m", c=8),
                    )

            return unit

        for jc in range(8):
            filler.append((("o", b, qb, jc), mk(jc)))

    # ---- Upfront: batch-0 K projection (all 4 blocks) + Q block 0; the rest
    # flows through the filler queue during batch-0 attention.
    for n in range(QB):
        emit_qk_block("k", n)
    emit_qk_block("q", 0)
    for n in range(1, QB):
        filler.append((("p", "q", n), lambda n=n: emit_qk_block("q", n)))
    for n in range(QB, 2 * QB):
        for name in ("k", "q"):
            filler.append(
                (("p", name, n), lambda name=name, n=n: emit_qk_block(name, n))
            )
    for rc in range(KC, 2 * KC):
        filler.append((("v", rc), lambda rc=rc: emit_v_chunk(rc)))

    # ---- Attention, software-pipelined: per k-chunk emit scores(kc),
    # exp(kc), one filler unit, attnV(kc-1).
    def emit_attnv(b, qb, kc, pt, acc0, acc1):
        vc = b * KC + kc
        nc.tensor.matmul(
            acc0[:],
            vaug0[:, vc * 65 : (vc + 1) * 65],
            pt[:, 0:512],
            start=(kc == 0),
            stop=(kc == KC - 1),
        )
        nc.tensor.matmul(
            acc1[:],
            vaug1[:, vc * W1 : (vc + 1) * W1],
            pt[:, 512:1024],
            start=(kc == 0),
            stop=(kc == KC - 1),
        )

    qT, kT = proj["q"], proj["k"]
    last = (B - 1, QB - 1)
    for b in range(B):
        outT = big.tile([128, S], BF, tag=f"outT{b}", name=f"outT{b}")
        for qb in range(QB):
            # deadline: projections this q-block's scores read must exist
            # before the reads are emitted (Tile deps follow emission order).
            for n in range(b * QB, (b + 1) * QB):
                ensure(("p", "k", n))
            ensure(("p", "q", b * QB + qb))
            q0 = b * S + qb * 512
            acc0 = ps_acc.tile([65, 512], F32, tag="acc0", name=f"acc0_{b}_{qb}")
            acc1 = ps_acc.tile([W1, 512], F32, tag="acc1", name=f"acc1_{b}_{qb}")
            pts = [None] * KC
            for kc in range(KC):
                k0 = b * S + kc * 128
                sc = ps_sc.tile([128, 1024], F32, tag="sc", name=f"sc{b}_{qb}_{kc}")
                for h in range(2):
                    nc.tensor.matmul(
                        sc[:, bass.ts(h, 512)],
                        kT[h * 64 : (h + 1) * 64, k0 : k0 + 128],
                        qT[h * 64 : (h + 1) * 64, q0 : q0 + 512],
                        start=True,
                        stop=True,
                    )
                pt = pt_pool.tile([128, 1024], BF, tag="pt", name=f"pt{b}_{qb}_{kc}")
                nc.scalar.activation(pt[:], sc[:], Exp, scale=0.125)
                pts[kc] = pt
                if DEBUG_DUMPS and b == 0 and qb == 0 and kc == 0:
                    dbgp = nc.dram_tensor("dbg_pt", [128, 1024], BF, kind="ExternalOutput")
                    nc.sync.dma_start(dbgp[:], pt[:])
                if b == 0 and qb == 0 and 2 * kc < KC:
                    # batch-0 V chunks are deadline-critical: chunk kc must
                    # exist before attnV(kc); 2 per k-chunk stays ahead.
                    emit_v_chunk(2 * kc)
                    emit_v_chunk(2 * kc + 1)
                else:
                    pump()
                if kc > 0:
                    ensure(("v", b * KC + kc - 1))
                    emit_attnv(b, qb, kc - 1, pts[kc - 1], acc0, acc1)
            ensure(("v", b * KC + KC - 1))
            emit_attnv(b, qb, KC - 1, pts[KC - 1], acc0, acc1)

            # ---- normalize ----
            # custom-DVE ops can't read PSUM on hardware: hop the denominator
            # rows through SBUF before the fast reciprocal.
            rec_bf = small.tile([W1, 512], BF, tag="rec_bf", name=f"recb{b}_{qb}")
            nc.vector.tensor_copy(den_t[64:65, :], acc0[64:65, :])
            nc.vector.tensor_copy(den_t[96:97, :], acc1[96:97, :])
            nc.vector.reciprocal_approx_fast(rec_t[:], den_t[:])
            nc.vector.tensor_copy(rec_bf[64:65, :], rec_t[64:65, :])
            nc.vector.tensor_copy(rec_bf[96:97, :], rec_t[96:97, :])

            def norm_post():
                # ISA: TensorTensor src0/src1 can't both be PSUM, so the
                # broadcast recip rows hop through SBUF before the muls.
                rep0 = ps_work.tile([64, 512], F32, tag="work", name=f"rep0_{b}_{qb}")
                nc.tensor.matmul(
                    rep0[:], ones_t[64:65, :], rec_bf[64:65, :],
                    start=True, stop=True, tile_position=(64, 0),
                )
                rep1 = ps_work.tile([64, 512], F32, tag="work", name=f"rep1_{b}_{qb}")
                nc.tensor.matmul(
                    rep1[:], ones_t[96:97, :], rec_bf[96:97, :],
                    start=True, stop=True, tile_position=(96, 0),
                )
                rep0_sb = small.tile([64, 512], F32, tag="rep0sb", name=f"r0s{b}_{qb}")
                rep1_sb = small.tile([64, 512], F32, tag="rep1sb", name=f"r1s{b}_{qb}")
                nc.vector.tensor_copy(rep0_sb[:], rep0[:])
                nc.vector.tensor_copy(rep1_sb[:], rep1[:])
                nc.vector.tensor_mul(
                    outT[0:64, bass.ts(qb, 512)], acc0[0:64, :], rep0_sb[:]
                )
                tmp = small.tile([64, 512], BF, tag="tmp", name=f"tmp{b}_{qb}")
                nc.vector.tensor_mul(tmp[:], acc1[0:64, :], rep1_sb[:])
                nc.gpsimd.dma_start(outT[64:128, bass.ts(qb, 512)], tmp[:])
                if DEBUG_DUMPS and b == 0 and qb == 0:
                    for dn, dt_ in (
                        ("rec", rec_t[64:65, :]),
                        ("rec1", rec_t[96:97, :]),
                        ("rep0", rep0_sb[:]),
                        ("rep1", rep1_sb[:]),
                        ("vaug0", vaug0[:, 0:65]),
                        ("vaug1", vaug1[:, 0:W1]),
                    ):
                        dbgt = nc.dram_tensor(
                            f"dbg_{dn}", list(dt_.shape), dt_.dtype, kind="ExternalOutput"
                        )
                        nc.sync.dma_start(dbgt[:], dt_)

            if DEBUG_DUMPS and qb == QB - 1:
                dbgo = nc.dram_tensor(f"dbg_outT{b}", [128, S], BF, kind="ExternalOutput")
                nc.sync.dma_start(dbgo[:], outT[:])
            if (b, qb) == last:
                # tail: normalize immediately, then drain all deferred work.
                norm_post()
                while filler:
                    pump()
                push_oproj(b, qb, outT)
                while filler:
                    pump()
            else:
                norm_post()
                push_oproj(b, qb, outT)


def kernel(x, mask, W_Q, W_K, W_V, W_O, _trace=False):
    # mask is all-ones for this problem; the reference `where(mask==0, -inf)` is a
    # no-op, so it is not shipped to the device.
    x = np.ascontiguousarray(np.asarray(x), dtype=np.float32)
    xT_bf = np.ascontiguousarray(np.asarray(x).reshape(BS, D).T).astype(BF16)

    if "nc" not in _nc_cache:
        _nc_cache["nc"] = _build_program()
    nc = _nc_cache["nc"]

    in_maps = []
    for c in range(N_CORES):
        sl = slice(c * SLW, (c + 1) * SLW)
        in_maps.append(
            {
                "xT": xT_bf,
                "wqT": np.ascontiguousarray(np.asarray(W_Q)[sl, :].T).astype(BF16),
                "wkT": np.ascontiguousarray(np.asarray(W_K)[sl, :].T).astype(BF16),
                "wvT": np.ascontiguousarray(np.asarray(W_V)[sl, :].T).astype(BF16),
                "woT": np.ascontiguousarray(np.asarray(W_O)[:, sl].T).astype(BF16),
            }
        )

    res = run_bass_kernel_spmd(nc, in_maps, core_ids=list(range(N_CORES)), trace=_trace)
    _nc_cache["last_result"] = res

    total = np.zeros((D, BS), dtype=np.float32)
    for c in range(N_CORES):
        total += res.results[c]["out"].astype(np.float32)
    return np.ascontiguousarray(total.T).reshape(B, S, D)


# revision 27
# speedup vs baseline: 1.0922x; 1.0922x over previous
"""Multi-head attention kernel for Trainium2, sharded over 8 NeuronCores.

Problem: B=2, S=2048, D=1024, H=16 heads (d_k=64), fp32 in/out, mask == all-ones.

Sharding: 2 heads per core (head/tensor parallel). Each core computes, for its
128-wide slice `sl` of the projection output dims:
    QT/KT = (W_*[sl] @ x.T)         -> [128, 4096]  (transposed layout, bf16)
    V     = x @ W_V[sl].T chunks    -> vaug tiles [krows, dims|ones]
    per (b, head): scoresT = K_h @ Q_h.T (contraction d_k=64, row-tiled pair)
    P.T = exp(scoresT / 8)          (no max-subtraction needed: |scores| < 8)
    acc_h = [V_h | 1].T @ P.T       -> psum; trailing row = softmax denom
    outT = acc * recip(denom) via fast-reciprocal + K=1 PE broadcast matmuls
    partialT = woT.T @ outT         -> partial.T [1024, 4096] bf16
Host: sums the 8 partials (fp32), transposes, reshapes.

Scheduling: scores/exp/attnV are software-pipelined (attnV one k-chunk behind
scores); independent matmul work (other-batch projections, V chunks, deferred
output projection) is pumped one unit per k-chunk to keep the PE continuously
busy while ACT computes exp.
"""
import sys

sys.path.insert(0, "/opt/trn_rl_repo")

from contextlib import ExitStack

import ml_dtypes
import numpy as np

import concourse.bass as bass
from concourse import bacc
import concourse.mybir as mybir
import concourse.tile as tile
from concourse.bass_utils import run_bass_kernel_spmd

BF16 = ml_dtypes.bfloat16
D = 1024
B = 2
S = 2048
BS = B * S            # 4096 rows
N_CORES = 8
SLW = D // N_CORES    # 128 output dims per core (2 heads x 64)
DK = 64
KC = S // 128         # 16 k-chunks per batch
QB = S // 512         # 4 q-blocks of 512 per batch
W1 = 97               # head-1 acc width: 64 data + 32 pad + denom at row 96
F32 = mybir.dt.float32
BF = mybir.dt.bfloat16

_nc_cache = {}
DEBUG_DUMPS = False  # when True, adds intermediate-dump outputs (dev only)


def _build_program():
    nc = bacc.Bacc("TRN2", target_bir_lowering=False, debug=False, num_devices=8)
    xT = nc.dram_tensor("xT", [D, BS], BF, kind="ExternalInput")
    wqT = nc.dram_tensor("wqT", [D, SLW], BF, kind="ExternalInput")
    wkT = nc.dram_tensor("wkT", [D, SLW], BF, kind="ExternalInput")
    wvT = nc.dram_tensor("wvT", [D, SLW], BF, kind="ExternalInput")
    woT = nc.dram_tensor("woT", [SLW, D], BF, kind="ExternalInput")
    out = nc.dram_tensor("out", [D, BS], BF, kind="ExternalOutput")

    with tile.TileContext(nc) as tc, ExitStack() as ctx:
        _emit(ctx, tc, nc, xT, wqT, wkT, wvT, woT, out)
    nc.compile()
    return nc


def _emit(ctx, tc, nc, xT, wqT, wkT, wvT, woT, out):
    Exp = mybir.ActivationFunctionType.Exp

    consts = ctx.enter_context(tc.tile_pool(name="consts", bufs=1))
    big = ctx.enter_context(tc.tile_pool(name="big", bufs=1))
    stage = ctx.enter_context(tc.tile_pool(name="stage", bufs=2))
    small = ctx.enter_context(tc.tile_pool(name="small", bufs=2))
    pt_pool = ctx.enter_context(tc.tile_pool(name="pt", bufs=3))
    # PSUM (8 banks): scores 2 x [128,1024] = 4 banks; attnV accumulators
    # (held across each q-block's k loop) 2 banks; shared work ring (V/proj
    # psum, output-proj matmuls, reciprocal broadcasts) 2 banks.
    ps_sc = ctx.enter_context(tc.tile_pool(name="ps_sc", bufs=2, space="PSUM"))
    ps_acc = ctx.enter_context(tc.tile_pool(name="ps_acc", bufs=1, space="PSUM"))
    ps_work = ctx.enter_context(tc.tile_pool(name="ps_work", bufs=2, space="PSUM"))

    # ---- Input DMAs: wk/wq first (first projections), x in 32 column-major
    # pieces spread over 4 engine queues so the first projection block is
    # ready ~3us in; wv/wo ordered later on queues that are past their
    # critical pieces.
    w_sbs = {}
    for name, w in (("k", wkT), ("q", wqT), ("v", wvT)):
        w_sbs[name] = consts.tile([128, 8 * SLW], BF, tag=f"w{name}", name=f"w{name}")
    nc.sync.dma_start(
        w_sbs["k"][:].rearrange("p (c m) -> p c m", c=8),
        wkT[:].rearrange("(c p) m -> p c m", p=128),
    )
    woT_sb = consts.tile([128, D], BF, tag="wo")

    xt_sb = [big.tile([128, BS], BF, tag=f"xt{c}", name=f"xt{c}") for c in range(8)]
    engines = [nc.sync, nc.scalar, nc.gpsimd]

    def xt_piece(c, p):
        cols = slice(p * 1024, (p + 1) * 1024)
        engines[c % 3].dma_start(xt_sb[c][:, cols], xT[c * 128 : (c + 1) * 128, cols])

    # Queue-priority order: wk + batch-0 pieces lead their queues so the
    # first k-projection block is ready ASAP; wq/wv/wo slot in after the
    # pieces they must not delay.
    for c in range(8):
        xt_piece(c, 0)
    nc.sync.dma_start(
        w_sbs["q"][:].rearrange("p (c m) -> p c m", c=8),
        wqT[:].rearrange("(c p) m -> p c m", p=128),
    )
    for c in range(8):
        xt_piece(c, 1)
    nc.scalar.dma_start(
        w_sbs["v"][:].rearrange("p (c m) -> p c m", c=8),
        wvT[:].rearrange("(c p) m -> p c m", p=128),
    )
    nc.sync.dma_start(woT_sb[:], woT[:])
    for p in (2, 3):
        for c in range(8):
            xt_piece(c, p)

    # ---- Constants: vaug ones-columns (memset 1.0; V copies overwrite data
    # cols, pad cols harmlessly stay 1), K=1 broadcast rows at partitions
    # 64 and 96 (legal PE tile_position rows).
    # memsets on DVE: it's idle at startup and doesn't carry input DMAs.
    vaug0 = consts.tile([128, KC * B * 65], BF, tag="vaug0", name="vaug0")
    vaug1 = consts.tile([128, KC * B * W1], BF, tag="vaug1", name="vaug1")
    nc.vector.memset(vaug0[:], 1.0)
    nc.vector.memset(vaug1[:], 1.0)
    ones_t = consts.tile([128, 64], BF, tag="ones_t", name="ones_t")
    nc.vector.memset(ones_t[64:65, :], 1.0)
    nc.vector.memset(ones_t[96:97, :], 1.0)
    # reciprocal_approx_fast mis-executes on single-partition slices on HW,
    # so the denominators ride in rows 64/96 of a full-height tile and the
    # reciprocal runs over all 128 partitions (same cycle count — cost is
    # free-dim size). Unused rows hold 1.0 so the op stays in-range.
    den_t = consts.tile([128, 512], F32, tag="den_t", name="den_t")
    rec_t = consts.tile([128, 512], F32, tag="rec_t", name="rec_t")
    nc.vector.memset(den_t[:], 1.0)

    proj = {}
    for name in ("k", "q"):
        proj[name] = big.tile([128, BS], BF, tag=f"{name}T", name=f"{name}T")

    def emit_qk_block(name, n):
        # n: global 512-col block index (0..7) over both batches.
        dst = proj[name]
        w_sb = w_sbs[name]
        ps = ps_work.tile([128, 512], F32, tag="work", name=f"ps_{name}{n}")
        for d in range(8):
            nc.tensor.matmul(
                ps[:],
                w_sb[:, bass.ts(d, SLW)],
                xt_sb[d][:, bass.ts(n, 512)],
                start=(d == 0),
                stop=(d == 7),
            )
        nc.vector.tensor_copy(dst[:, bass.ts(n, 512)], ps[:])

    def emit_v_chunk(rc):
        # rc: global 128-row chunk (0..31). vaug0[:, rc*65+m] = V[rc*128+p, m],
        # vaug1[:, rc*97+m] = V[rc*128+p, 64+m]; ones cols feed the denom rows.
        wv_sb = w_sbs["v"]
        ps = ps_work.tile([128, 128], F32, tag="work", name=f"ps_v{rc}")
        for d in range(8):
            nc.tensor.matmul(
                ps[:],
                xt_sb[d][:, bass.ts(rc, 128)],
                wv_sb[:, bass.ts(d, SLW)],
                start=(d == 0),
                stop=(d == 7),
            )
        # GPSIMD can't access PSUM, so these casts stay on DVE.
        nc.vector.tensor_copy(vaug0[:, rc * 65 : rc * 65 + 64], ps[:, 0:64])
        nc.vector.tensor_copy(vaug1[:, rc * W1 : rc * W1 + 64], ps[:, 64:128])

    # ---- Filler units: independent PE work pumped one unit per k-chunk so
    # the PE never head-of-line blocks on ACT's exp. Units carry keys so
    # consumers can force ("ensure") their producers to emit first.
    filler = []  # list of (key, fn)

    def pump():
        if filler:
            filler.pop(0)[1]()

    def ensure(key):
        for i, (k, fn) in enumerate(filler):
            if k == key:
                filler.pop(i)
                fn()
                return

    # ---- Output projection, one q-block of one batch = 8 jc units + 1 DMA.
    # In tail mode (final q-block) the psum->sbuf casts alternate DVE/ACT
    # (both otherwise idle) and the store is split so DMA overlaps the casts.
    Copy = mybir.ActivationFunctionType.Copy

    def push_oproj(b, qb, outT, tail=False):
        st = stage.tile([128, 8 * 512], BF, tag="st", name=f"st{b}_{qb}")

        def store(jc_lo, jc_hi, eng):
            cols = slice(b * S + qb * 512 + 0, b * S + (qb + 1) * 512)
            eng.dma_start(
                out[:, cols]
                .rearrange("(c p) m -> p c m", p=128)[:, jc_lo:jc_hi, :],
                st[:].rearrange("p (c m) -> p c m", c=8)[:, jc_lo:jc_hi, :],
            )

        def mk(jc):
            def unit():
                pm = ps_work.tile([128, 512], F32, tag="work", name=f"pm{b}_{qb}_{jc}")
                nc.tensor.matmul(
                    pm[:],
                    woT_sb[:, bass.ts(jc, 128)],
                    outT[:, bass.ts(qb, 512)],
                    start=True,
                    stop=True,
                )
                if tail and jc % 2 == 1:
                    nc.scalar.activation(st[:, bass.ts(jc, 512)], pm[:], Copy)
                else:
                    nc.vector.tensor_copy(st[:, bass.ts(jc, 512)], pm[:])
                if tail and jc == 3:
                    store(0, 4, nc.sync)
                elif jc == 7:
                    if tail:
                        store(4, 8, nc.scalar)
                    else:
                        store(0, 8, nc.sync)

            return unit

        for jc in range(8):
            filler.append((("o", b, qb, jc), mk(jc)))

    # ---- Upfront: batch-0 K projection (all 4 blocks) + Q block 0; the rest
    # flows through the filler queue during batch-0 attention.
    for n in range(QB):
        emit_qk_block("k", n)
    emit_qk_block("q", 0)
    for n in range(1, QB):
        filler.append((("p", "q", n), lambda n=n: emit_qk_block("q", n)))
    for n in range(QB, 2 * QB):
        for name in ("k", "q"):
            filler.append(
                (("p", name, n), lambda name=name, n=n: emit_qk_block(name, n))
            )
    for rc in range(KC, 2 * KC):
        filler.append((("v", rc), lambda rc=rc: emit_v_chunk(rc)))

    # ---- Attention, software-pipelined: per k-chunk emit scores(kc),
    # exp(kc), one filler unit, attnV(kc-1).
    def emit_attnv(b, qb, kc, pt, acc0, acc1):
        vc = b * KC + kc
        nc.tensor.matmul(
            acc0[:],
            vaug0[:, vc * 65 : (vc + 1) * 65],
            pt[:, 0:512],
            start=(kc == 0),
            stop=(kc == KC - 1),
        )
        nc.tensor.matmul(
            acc1[:],
            vaug1[:, vc * W1 : (vc + 1) * W1],
            pt[:, 512:1024],
            start=(kc == 0),
            stop=(kc == KC - 1),
        )

    qT, kT = proj["q"], proj["k"]
    last = (B - 1, QB - 1)
    for b in range(B):
        outT = big.tile([128, S], BF, tag=f"outT{b}", name=f"outT{b}")
        for qb in range(QB):
            # deadline: projections this q-block's scores read must exist
            # before the reads are emitted (Tile deps follow emission order).
            for n in range(b * QB, (b + 1) * QB):
                ensure(("p", "k", n))
            ensure(("p", "q", b * QB + qb))
            q0 = b * S + qb * 512
            acc0 = ps_acc.tile([65, 512], F32, tag="acc0", name=f"acc0_{b}_{qb}")
            acc1 = ps_acc.tile([W1, 512], F32, tag="acc1", name=f"acc1_{b}_{qb}")
            pts = [None] * KC
            for kc in range(KC):
                k0 = b * S + kc * 128
                sc = ps_sc.tile([128, 1024], F32, tag="sc", name=f"sc{b}_{qb}_{kc}")
                for h in range(2):
                    nc.tensor.matmul(
                        sc[:, bass.ts(h, 512)],
                        kT[h * 64 : (h + 1) * 64, k0 : k0 + 128],
                        qT[h * 64 : (h + 1) * 64, q0 : q0 + 512],
                        start=True,
                        stop=True,
                    )
                pt = pt_pool.tile([128, 1024], BF, tag="pt", name=f"pt{b}_{qb}_{kc}")
                nc.scalar.activation(pt[:], sc[:], Exp, scale=0.125)
                pts[kc] = pt
                if DEBUG_DUMPS and b == 0 and qb == 0 and kc == 0:
                    dbgp = nc.dram_tensor("dbg_pt", [128, 1024], BF, kind="ExternalOutput")
                    nc.sync.dma_start(dbgp[:], pt[:])
                if b == 0 and qb == 0 and 2 * kc < KC:
                    # batch-0 V chunks are deadline-critical: chunk kc must
                    # exist before attnV(kc); 2 per k-chunk stays ahead.
                    emit_v_chunk(2 * kc)
                    emit_v_chunk(2 * kc + 1)
                else:
                    pump()
                if kc > 0:
                    ensure(("v", b * KC + kc - 1))
                    emit_attnv(b, qb, kc - 1, pts[kc - 1], acc0, acc1)
            ensure(("v", b * KC + KC - 1))
            emit_attnv(b, qb, KC - 1, pts[KC - 1], acc0, acc1)

            # ---- normalize ----
            # custom-DVE ops can't read PSUM on hardware: hop the denominator
            # rows through SBUF before the fast reciprocal. The acc data rows
            # also hop to SBUF immediately — this frees the acc PSUM banks for
            # the next q-block and satisfies the TensorTensor one-PSUM rule
            # (the muls then read sbuf-acc x psum-rep).
            rec_bf = small.tile([W1, 512], BF, tag="rec_bf", name=f"recb{b}_{qb}")
            a0_sb = small.tile([64, 512], F32, tag="a0sb", name=f"a0s{b}_{qb}")
            a1_sb = small.tile([64, 512], F32, tag="a1sb", name=f"a1s{b}_{qb}")
            nc.vector.tensor_copy(den_t[64:65, :], acc0[64:65, :])
            nc.vector.tensor_copy(den_t[96:97, :], acc1[96:97, :])
            nc.vector.tensor_copy(a0_sb[:], acc0[0:64, :])
            nc.vector.tensor_copy(a1_sb[:], acc1[0:64, :])
            nc.vector.reciprocal_approx_fast(rec_t[:], den_t[:])
            nc.vector.tensor_copy(rec_bf[64:65, :], rec_t[64:65, :])
            nc.vector.tensor_copy(rec_bf[96:97, :], rec_t[96:97, :])

            def norm_post():
                rep0 = ps_work.tile([64, 512], F32, tag="work", name=f"rep0_{b}_{qb}")
                nc.tensor.matmul(
                    rep0[:], ones_t[64:65, :], rec_bf[64:65, :],
                    start=True, stop=True, tile_position=(64, 0),
                )
                rep1 = ps_work.tile([64, 512], F32, tag="work", name=f"rep1_{b}_{qb}")
                nc.tensor.matmul(
                    rep1[:], ones_t[96:97, :], rec_bf[96:97, :],
                    start=True, stop=True, tile_position=(96, 0),
                )
                nc.vector.tensor_mul(
                    outT[0:64, bass.ts(qb, 512)], a0_sb[:], rep0[:]
                )
                tmp = small.tile([64, 512], BF, tag="tmp", name=f"tmp{b}_{qb}")
                nc.vector.tensor_mul(tmp[:], a1_sb[:], rep1[:])
                nc.sync.dma_start(outT[64:128, bass.ts(qb, 512)], tmp[:])
                if DEBUG_DUMPS and b == 0 and qb == 0:
                    for dn, dt_ in (
                        ("rec", rec_t[64:65, :]),
                        ("rec1", rec_t[96:97, :]),
                        ("rep0", a0_sb[:]),
                        ("rep1", a1_sb[:]),
                        ("vaug0", vaug0[:, 0:65]),
                        ("vaug1", vaug1[:, 0:W1]),
                    ):
                        dbgt = nc.dram_tensor(
                            f"dbg_{dn}", list(dt_.shape), dt_.dtype, kind="ExternalOutput"
                        )
                        nc.sync.dma_start(dbgt[:], dt_)

            if DEBUG_DUMPS and qb == QB - 1:
                dbgo = nc.dram_tensor(f"dbg_outT{b}", [128, S], BF, kind="ExternalOutput")
                nc.sync.dma_start(dbgo[:], outT[:])
            if (b, qb) == last:
                # tail: normalize immediately, then drain all deferred work.
                norm_post()
                while filler:
                    pump()
                push_oproj(b, qb, outT, tail=True)
                while filler:
                    pump()
            else:
                norm_post()
                push_oproj(b, qb, outT)


def kernel(x, mask, W_Q, W_K, W_V, W_O, _trace=False):
    # mask is all-ones for this problem; the reference `where(mask==0, -inf)` is a
    # no-op, so it is not shipped to the device.
    x = np.ascontiguousarray(np.asarray(x), dtype=np.float32)
    xT_bf = np.ascontiguousarray(np.asarray(x).reshape(BS, D).T).astype(BF16)

    if "nc" not in _nc_cache:
        _nc_cache["nc"] = _build_program()
    nc = _nc_cache["nc"]

    in_maps = []
    for c in range(N_CORES):
        sl = slice(c * SLW, (c + 1) * SLW)
        in_maps.append(
            {
                "xT": xT_bf,
                "wqT": np.ascontiguousarray(np.asarray(W_Q)[sl, :].T).astype(BF16),
                "wkT": np.ascontiguousarray(np.asarray(W_K)[sl, :].T).astype(BF16),
                "wvT": np.ascontiguousarray(np.asarray(W_V)[sl, :].T).astype(BF16),
                "woT": np.ascontiguousarray(np.asarray(W_O)[:, sl].T).astype(BF16),
            }
        )

    res = run_bass_kernel_spmd(nc, in_maps, core_ids=list(range(N_CORES)), trace=_trace)
    _nc_cache["last_result"] = res

    total = np.zeros((D, BS), dtype=np.float32)
    for c in range(N_CORES):
        total += res.results[c]["out"].astype(np.float32)
    return np.ascontiguousarray(total.T).reshape(B, S, D)
